# revision 17
# baseline (speedup 1.0000x reference)
"""CNNSummarizer (CNN encoder + 2-layer LSTM decoder + vocab projection) on 8 trn2 cores.

Sharding:
  - encoder: data-parallel over batch (4 batches per core); one AllGather of the
    per-batch encoder contribution to the LSTM-1 input preactivation (32KB).
  - LSTM recurrence: replicated on all 8 cores (small-collective latency makes
    per-step sharding a loss).
  - vocab projection (H -> V GEMM): column-sharded, 4000 vocab per core.

v2 changes vs the original working version:
  - all-bf16 data path (embeddings pre-cast on host, conv / fc / Xdec / bypass
    matmuls bf16) -- avoids fp32-HIGH 2-pass matmuls and halves DMA traffic.
  - recurrence + vocab weights DMA'd at program start (pools coexist with the
    encoder pools) so the recurrence never waits on weight loads.
  - index DMAs issued first so embedding gathers start immediately.
  - dec-token gathers all issued before the AllGather on the gpsimd queue.
  - Xdec m-chunks 0..3 computed during the AllGather window; chunks 4..15
    interleaved into recurrence steps 0..11 as PE filler work.
  - greedy vocab-unit emission (fills PE gaps while LSTM cell chains run on
    ACT/DVE, keeping the PE HAM clock-gate warm).

The LSTM input preactivations X@w_ih1 are precomputed for all steps (teacher
forcing) as one big GEMM into a DRAM buffer packed as (t, 32*gateblock+batch,
512); the recurrent h@w_hh GEMMs then run as 4-way column-tiled matmuls
(stationary = h^T K-chunk, streaming = weight rows) producing gates directly in
a (gateblock*32+b, d) PSUM layout, gate order host-permuted to [i, f, o, g] so
sigmoid runs as one 96-partition op.
"""

import math
from contextlib import ExitStack

import numpy as np

import concourse.bacc as bacc
import concourse.bass as bass
import concourse.mybir as mybir
import concourse.tile as tile
from concourse.masks import make_identity

V, E, H, F = 32000, 256, 512, 256
FS = (3, 4, 5)
B, S, T = 32, 512, 64
NCORES = 8
BL = B // NCORES          # batches per core
VS = V // NCORES          # vocab shard per core
TT = T - 1                # decode steps actually computed
G4 = 4 * H                # 2048 gates

dt = mybir.dt
F32 = dt.float32
BF16 = dt.bfloat16
AF = mybir.ActivationFunctionType
ALU = mybir.AluOpType
AX = mybir.AxisListType


def build(tt=TT, vs=VS, trace_sim=False):
    """Build the per-core program. All 8 cores run the same NEFF; sharding comes
    from per-core input values."""
    R = tt * B                       # rows of the (t, b) decode matrix
    NM = math.ceil(R / 128)          # m-chunks of decode rows
    NCH = NM                         # dec-token gather chunks (128 tokens each)
    RPAD = NM * 128
    NV = math.ceil(vs / 512)
    XDEC_PRE = 12                    # Xdec chunks computed before the recurrence

    nc = bacc.Bacc("TRN2", target_bir_lowering=False, debug=False,
                   num_devices=NCORES)

    def inp(name, shape, dtype=BF16):
        return nc.dram_tensor(name, list(shape), dtype, kind="ExternalInput").ap()

    src_idx = inp("src_idx", (128, (BL * S) // 128), dt.int32)
    dec_idx = inp("dec_idx", (128, NCH), dt.int32)
    enc_emb = inp("enc_emb", (V, E))          # bf16 (host pre-cast)
    dec_emb = inp("dec_emb", (V, E))          # bf16
    wconv = {k: inp(f"wconv{k}", (128, k * 4 * 128)) for k in FS}
    bconv = inp("bconv", (128, 2 * len(FS)), F32)   # col = fc*3 + k_idx
    fc1T = inp("fc1T", (128, 6 * H))
    fc1b = inp("fc1b", (1, H))
    fc2T = inp("fc2T", (128, 4 * H))
    fc2b = inp("fc2b", (1, H))
    WdT = inp("WdT", (128, 2 * G4))
    WeT = inp("WeT", (128, 4 * G4))
    b1row = inp("b1row", (1, G4))
    b2pack = inp("b2pack", (128, H))
    whh1T = inp("whh1T", (128, 4 * G4))
    wih2T = inp("wih2T", (128, 4 * G4))
    whh2T = inp("whh2T", (128, 4 * G4))
    owT = inp("owT", (128, 4 * vs))
    obcast_in = inp("obcast", (128, vs), F32)

    out_dram = nc.dram_tensor("logits_sh", [R, vs], F32,
                              kind="ExternalOutput").ap()

    with tile.TileContext(nc, trace_sim=trace_sim) as tc:
        with ExitStack() as ctx:
            dram = ctx.enter_context(tc.tile_pool(name="dram", bufs=1,
                                                  space="DRAM"))
            xih_dram = dram.tile([tt, 128, H], BF16)
            cc_in = dram.tile([BL, G4], F32)
            cc_out = dram.tile([B, G4], F32, addr_space="Shared")

            # ---- constants ----
            const = ctx.enter_context(tc.tile_pool(name="const", bufs=1))
            identF = const.tile([128, 128], F32)
            make_identity(nc, identF[:])
            ident_bf = const.tile([128, 128], BF16)
            nc.vector.tensor_copy(ident_bf[:], identF[:])
            scrF = const.tile([128, 128], F32)
            nc.vector.memset(scrF[:], 0.0)
            zpad = const.tile([128, 8], BF16)
            nc.vector.tensor_copy(zpad[:], scrF[:, 0:8])
            nc.vector.memset(scrF[0:1, :], 1.0)
            ones_bf = const.tile([1, 128], BF16)
            nc.vector.tensor_copy(ones_bf[:], scrF[0:1, :])

            # ---- persistent recurrence weights: DMA'd up front ----
            rw = ctx.enter_context(tc.tile_pool(name="rw", bufs=1))
            whh1_sb = rw.tile([128, 4 * G4], BF16)
            wih2_sb = rw.tile([128, 4 * G4], BF16)
            whh2_sb = rw.tile([128, 4 * G4], BF16)
            b2_sb = rw.tile([128, H], BF16)
            xeall_f32 = rw.tile([128, H], F32)
            xeall_bf = rw.tile([128, H], BF16)

            # h2^T lives across recurrence + vocab phases
            big = ctx.enter_context(tc.tile_pool(name="big", bufs=1))
            h2T_all = big.tile([128, 4 * RPAD], BF16)   # [kc] blocks of h2^T

            # ---- mid-lived: dec-token gather + Xdec staging ----
            midp = ctx.enter_context(tc.tile_pool(name="midp", bufs=1))
            idx_d_sb = midp.tile([128, NCH], dt.int32)
            dembT_sb = midp.tile([128, 2 * RPAD], BF16)   # [ec] blocks
            WdT_sb = midp.tile([128, 2 * G4], BF16)
            gpool_dec = ctx.enter_context(tc.tile_pool(name="gpool_dec",
                                                       bufs=NM))
            xdpool = ctx.enter_context(tc.tile_pool(name="xdpool", bufs=4))

            # =========================================================
            # Phase 1: encoder (my BL batches) + Xdec chunks 0..3
            # =========================================================
            p1 = ExitStack()
            encp = p1.enter_context(tc.tile_pool(name="encp", bufs=1))
            gpool = p1.enter_context(tc.tile_pool(name="gpool", bufs=16))
            tpp = p1.enter_context(tc.tile_pool(name="tpp", bufs=2,
                                                space="PSUM"))
            cps = p1.enter_context(tc.tile_pool(name="cps", bufs=3,
                                                space="PSUM"))
            fps = p1.enter_context(tc.tile_pool(name="fps", bufs=2,
                                                space="PSUM"))

            wconv_sb = {k: encp.tile([128, k * 4 * 128], BF16,
                                     name=f"wconv{k}_sb") for k in FS}
            bconv_sb = encp.tile([128, 2 * len(FS)], F32)
            fc1T_sb = encp.tile([128, 6 * H], BF16)
            fc2T_sb = encp.tile([128, 4 * H], BF16)
            fc1b_sb = encp.tile([1, H], BF16)
            fc2b_sb = encp.tile([1, H], BF16)
            WeT_sb = encp.tile([128, 4 * G4], BF16)
            b1_sb = encp.tile([1, G4], BF16)
            idx_s_sb = encp.tile([128, (BL * S) // 128], dt.int32)

            XPAD = BL * (S + 8)
            SEG = S + 8
            xT_sb = encp.tile([128, 2 * XPAD], BF16)        # [ec] blocks

            # ---- DMA issue order = priority order ----
            nc.sync.dma_start(idx_s_sb[:], src_idx)
            nc.sync.dma_start(idx_d_sb[:], dec_idx)
            for k in FS:
                nc.sync.dma_start(wconv_sb[k][:], wconv[k])
            nc.sync.dma_start(bconv_sb[:], bconv)
            nc.sync.dma_start(fc1T_sb[:], fc1T)
            nc.sync.dma_start(fc1b_sb[:], fc1b)
            nc.sync.dma_start(fc2T_sb[:], fc2T)
            nc.sync.dma_start(fc2b_sb[:], fc2b)
            nc.sync.dma_start(WdT_sb[:], WdT)
            nc.sync.dma_start(WeT_sb[:], WeT)
            nc.sync.dma_start(b1_sb[:], b1row)
            nc.sync.dma_start(b2_sb[:], b2pack)
            nc.sync.dma_start(whh1_sb[:], whh1T)
            nc.sync.dma_start(wih2_sb[:], wih2T)
            nc.sync.dma_start(whh2_sb[:], whh2T)

            def evict(dst, src, parity):
                if parity % 2 == 0:
                    nc.vector.tensor_copy(dst, src)
                else:
                    nc.scalar.copy(dst, src)

            # ---- src gathers (gpsimd queue) ----
            gts = []
            for b in range(BL):
                for ch in range(4):
                    gt = gpool.tile([128, E], BF16, tag="gath")
                    nc.gpsimd.indirect_dma_start(
                        out=gt[:], out_offset=None, in_=enc_emb,
                        in_offset=bass.IndirectOffsetOnAxis(
                            ap=idx_s_sb[:, 4 * b + ch:4 * b + ch + 1],
                            axis=0))
                    gts.append(gt)
            # ---- dec gathers, all prefetched before the AllGather ----
            gtds = []
            for m in range(NM):
                gt = gpool_dec.tile([128, E], BF16, tag="gathd")
                nc.gpsimd.indirect_dma_start(
                    out=gt[:], out_offset=None, in_=dec_emb,
                    in_offset=bass.IndirectOffsetOnAxis(
                        ap=idx_d_sb[:, m:m + 1], axis=0))
                gtds.append(gt)

            pooled = encp.tile([128, 6 * BL], BF16)

            def conv_batch(b):
                for ki, k in enumerate(FS):
                    for fc in range(2):
                        ps = cps.tile([128, 512], F32, tag="conv",
                                      space="PSUM")
                        first = True
                        for j in range(k):
                            for ec in range(2):
                                lhs = wconv_sb[k][
                                    :, (j * 4 + ec * 2 + fc) * 128:
                                    (j * 4 + ec * 2 + fc) * 128 + 128]
                                rhs = xT_sb[:, ec * XPAD + SEG * b + j:
                                            ec * XPAD + SEG * b + j + 512]
                                nc.tensor.matmul(
                                    ps[:], lhs, rhs, start=first,
                                    stop=(j == k - 1 and ec == 1))
                                first = False
                        kc = ki * 2 + fc
                        nc.vector.tensor_reduce(
                            pooled[:, BL * kc + b: BL * kc + b + 1],
                            ps[:, 0:S - k + 1], axis=AX.X, op=ALU.max)

            # conv staircase: transpose batch b's gathers, then its convs
            for b in range(BL):
                for ch in range(4):
                    gt = gts[4 * b + ch]
                    for ec in range(2):
                        tp = tpp.tile([128, 128], BF16, tag="tp",
                                      space="PSUM")
                        nc.tensor.transpose(
                            tp[:], gt[:, 128 * ec:128 * ec + 128],
                            ident_bf[:])
                        evict(xT_sb[:, ec * XPAD + SEG * b + 128 * ch:
                                    ec * XPAD + SEG * b + 128 * ch + 128],
                              tp[:], ch + ec)
                for ec in range(2):
                    nc.vector.tensor_copy(
                        xT_sb[:, ec * XPAD + SEG * b + S:
                              ec * XPAD + SEG * (b + 1)], zpad[:])
                conv_batch(b)

            for ki in range(len(FS)):
                for fc in range(2):
                    kc = ki * 2 + fc
                    nc.scalar.activation(
                        pooled[:, BL * kc: BL * kc + BL],
                        pooled[:, BL * kc: BL * kc + BL],
                        AF.Relu, bias=bconv_sb[:, fc * 3 + ki: fc * 3 + ki + 1])

            # ---- fc1 -> relu -> fc2 -> Xenc -> AllGather ----
            ps1 = fps.tile([BL, H], F32, tag="f", space="PSUM")
            for kc in range(6):
                nc.tensor.matmul(ps1[:], pooled[:, BL * kc: BL * kc + BL],
                                 fc1T_sb[:, H * kc: H * kc + H],
                                 start=(kc == 0), stop=False)
            nc.tensor.matmul(ps1[:], ones_bf[0:1, 0:BL], fc1b_sb[:],
                             start=False, stop=True)
            h1e = encp.tile([BL, H], BF16)
            nc.scalar.activation(h1e[:], ps1[:], AF.Relu)

            h1eT = encp.tile([128, 4 * BL], BF16)
            for kc in range(4):
                tp = tpp.tile([128, 128], BF16, tag="tp", space="PSUM")
                nc.tensor.transpose(tp[0:128, 0:BL],
                                    h1e[:, 128 * kc:128 * kc + 128],
                                    ident_bf[0:BL, 0:BL])
                nc.vector.tensor_copy(h1eT[:, BL * kc:BL * kc + BL],
                                      tp[0:128, 0:BL])

            ps2 = fps.tile([BL, H], F32, tag="f", space="PSUM")
            for kc in range(4):
                nc.tensor.matmul(ps2[:], h1eT[:, BL * kc:BL * kc + BL],
                                 fc2T_sb[:, H * kc:H * kc + H],
                                 start=(kc == 0), stop=False)
            nc.tensor.matmul(ps2[:], ones_bf[0:1, 0:BL], fc2b_sb[:],
                             start=False, stop=True)
            enc_sb = encp.tile([BL, H], BF16)
            nc.vector.tensor_copy(enc_sb[:], ps2[:])

            encT = encp.tile([128, 4 * BL], BF16)
            for kc in range(4):
                tp = tpp.tile([128, 128], BF16, tag="tp", space="PSUM")
                nc.tensor.transpose(tp[0:128, 0:BL],
                                    enc_sb[:, 128 * kc:128 * kc + 128],
                                    ident_bf[0:BL, 0:BL])
                nc.vector.tensor_copy(encT[:, BL * kc:BL * kc + BL],
                                      tp[0:128, 0:BL])

            xe_sb = encp.tile([BL, G4], F32)
            for n in range(4):
                ps = fps.tile([BL, 512], F32, tag="f", space="PSUM")
                for kc in range(4):
                    nc.tensor.matmul(
                        ps[:], encT[:, BL * kc:BL * kc + BL],
                        WeT_sb[:, kc * G4 + 512 * n:
                               kc * G4 + 512 * n + 512],
                        start=(kc == 0), stop=False)
                nc.tensor.matmul(ps[:], ones_bf[0:1, 0:BL],
                                 b1_sb[:, 512 * n:512 * n + 512],
                                 start=False, stop=True)
                nc.vector.tensor_copy(xe_sb[:, 512 * n:512 * n + 512], ps[:])
            nc.sync.dma_start(cc_in[:], xe_sb[:])

            nc.gpsimd.collective_compute(
                "AllGather", ALU.bypass,
                replica_groups=[list(range(NCORES))],
                ins=[cc_in.opt()], outs=[cc_out.opt()])

            # ---- Xdec chunk machinery (shared by phase 1 + recurrence) ----
            def xdec_chunk(m, tp_pool, tp_tag, ps_pool, ps_tag):
                tm = min(4, tt - 4 * m)
                Mm = 32 * tm
                gt = gtds[m]
                for ec in range(2):
                    tp = tp_pool.tile([128, 128], BF16, tag=tp_tag,
                                      space="PSUM")
                    nc.tensor.transpose(
                        tp[:], gt[:, 128 * ec:128 * ec + 128], ident_bf[:])
                    evict(dembT_sb[:, ec * RPAD + 128 * m:
                                   ec * RPAD + 128 * m + 128],
                          tp[:], m + ec)
                xd_sb = xdpool.tile([128, G4], BF16, tag="xd_sb")
                for n in range(4):
                    ps = ps_pool.tile([128, 512], F32, tag=ps_tag,
                                      space="PSUM")
                    for ec in range(2):
                        nc.tensor.matmul(
                            ps[0:Mm, :],
                            dembT_sb[:, ec * RPAD + 128 * m:
                                     ec * RPAD + 128 * m + Mm],
                            WdT_sb[:, ec * G4 + 512 * n:
                                   ec * G4 + 512 * n + 512],
                            start=(ec == 0), stop=(ec == 1))
                    evict(xd_sb[0:Mm, 512 * n:512 * n + 512],
                          ps[0:Mm, :], m + n)
                for tau in range(tm):
                    dst = xih_dram[4 * m + tau].rearrange(
                        "(j b) d -> b j d", j=4)
                    # scalar-queue DMA: the sync queue head-blocks on the
                    # xe-gated cc_in transfer during the AllGather window
                    nc.scalar.dma_start(dst, xd_sb[32 * tau:32 * tau + 32, :])

            # chunks 0..XDEC_PRE-1 overlap the AllGather
            for m in range(XDEC_PRE):
                xdec_chunk(m, tpp, "tp", cps, "conv")

            p1.close()

            # =========================================================
            # Phase 2: recurrence with packed gate PSUM, col-tiled GEMMs
            # gate-block order [i, f, o, g] on psum partitions [0:32,...]
            # =========================================================
            with ExitStack() as p2:
                vocabw = p2.enter_context(tc.tile_pool(name="vocabw", bufs=1))
                owT_sb = vocabw.tile([128, 4 * vs], BF16)
                obcast = vocabw.tile([128, vs], F32)
                # vector-queue DMAs: the sync queue head-blocks on the
                # AllGather-dependent loads below; these must not wait.
                nc.scalar.dma_start(owT_sb[:], owT)
                nc.scalar.dma_start(obcast[:], obcast_in)

                rp = p2.enter_context(tc.tile_pool(name="rp", bufs=2))
                xp = p2.enter_context(tc.tile_pool(name="xp", bufs=3))
                vo = p2.enter_context(tc.tile_pool(name="vo", bufs=3))
                rps = p2.enter_context(tc.tile_pool(name="rps", bufs=2,
                                                    space="PSUM"))
                dps = p2.enter_context(tc.tile_pool(name="dps", bufs=1,
                                                    space="PSUM"))
                tps = p2.enter_context(tc.tile_pool(name="tps", bufs=1,
                                                    space="PSUM"))
                vps = p2.enter_context(tc.tile_pool(name="vps", bufs=3,
                                                    space="PSUM"))

                # Xenc contribution, repacked to the (32*gateblock+b, d) layout
                nc.sync.dma_start(xeall_f32[:],
                                  cc_out.rearrange("b (j d) -> j b d", j=4))
                nc.vector.tensor_copy(xeall_bf[:], xeall_f32[:])

                dummy_ps = dps.tile([128, 512], F32, tag="d", space="PSUM")

                def dummy(n=256, count=1):
                    """Keep-warm matmuls: PE-busy filler during cell-chain
                    stalls so the HAM clock gate stays open."""
                    for _ in range(count):
                        nc.tensor.matmul(dummy_ps[:, 0:n], ident_bf[:],
                                         b2_sb[:, 0:n], start=True, stop=True,
                                         skip_group_check=True)

                c1 = rp.tile([64, H], BF16, tag="c1")
                nc.vector.memset(c1[32:64, :], 0.0)
                c2 = rp.tile([64, H], BF16, tag="c2")
                nc.vector.memset(c2[32:64, :], 0.0)

                def cell(ps_g, c_prev, tag):
                    """LSTM cell from packed-gate psum (128, H) -> (h, c_new).

                    Gate blocks keep their partition homes: i/f/o from one
                    96-partition sigmoid, tanh(g) lands at [0:32]. c lives at
                    [32:64]. All elementwise math in bf16.
                    """
                    sig = rp.tile([96, H], BF16, tag=f"sig{tag}")
                    nc.scalar.activation(sig[:], ps_g[0:96, :], AF.Sigmoid)
                    tg = rp.tile([32, H], BF16, tag=f"tg{tag}")
                    nc.scalar.activation(tg[:], ps_g[96:128, :], AF.Tanh)
                    c_new = rp.tile([64, H], BF16, tag=f"c{tag}")
                    nc.vector.tensor_mul(c_new[32:64, :], sig[32:64, :],
                                         c_prev[32:64, :])
                    m1 = rp.tile([64, H], BF16, tag=f"m1{tag}")
                    nc.vector.tensor_mul(m1[32:64, :], sig[0:32, :], tg[:])
                    nc.vector.tensor_add(c_new[32:64, :], m1[32:64, :],
                                         c_new[32:64, :])
                    th = rp.tile([96, H], BF16, tag=f"th{tag}")
                    nc.scalar.activation(th[64:96, :], c_new[32:64, :], AF.Tanh)
                    h = rp.tile([32, H], BF16, tag=f"h{tag}")
                    nc.vector.tensor_mul(h[:], sig[64:96, :], th[64:96, :])
                    return h, c_new

                def transpose_state(h, dsts):
                    tp = tps.tile([128, 128], BF16, tag="tps",
                                  space="PSUM")
                    for kc in range(4):
                        nc.tensor.transpose(tp[:, 32 * kc:32 * kc + 32],
                                            h[:, 128 * kc:128 * kc + 128],
                                            ident_bf[0:32, 0:32])
                    for dst in dsts:
                        nc.vector.tensor_copy(dst, tp[:])
                    return tp

                def gemm_block(ps, stat, stat_base, w_sb, final):
                    for kc in range(4):
                        for j in range(4):
                            nc.tensor.matmul(
                                ps[32 * j:32 * j + 32, :],
                                stat[:, stat_base(kc): stat_base(kc) + 32],
                                w_sb[:, kc * G4 + 512 * j:
                                     kc * G4 + 512 * j + 512],
                                start=False,
                                stop=(final and kc == 3 and j == 3),
                                skip_group_check=True,
                                tile_position=(0, 32 * j))

                def vocab_unit(m, n):
                    """One (row-chunk, vocab-tile) unit of the output GEMM."""
                    Mm = min(128, R - 128 * m)
                    nw = min(512, vs - 512 * n)
                    ps = vps.tile([128, 512], F32, tag="vps", space="PSUM")
                    for kc in range(4):
                        nc.tensor.matmul(
                            ps[0:Mm, 0:nw],
                            h2T_all[:, kc * RPAD + 128 * m:
                                    kc * RPAD + 128 * m + Mm],
                            owT_sb[:, kc * vs + 512 * n: kc * vs + 512 * n + nw],
                            start=(kc == 0), stop=(kc == 3))
                    ob = vo.tile([128, 512], F32, tag="ob")
                    nc.vector.tensor_add(ob[0:Mm, 0:nw], ps[0:Mm, 0:nw],
                                         obcast[0:Mm, 512 * n:512 * n + nw])
                    nc.sync.dma_start(
                        out_dram[128 * m:128 * m + Mm, 512 * n:512 * n + nw],
                        ob[0:Mm, 0:nw])

                vunits = [(m, n) for m in range(NM) for n in range(NV)]
                vemitted = 0

                h2T_view = h2T_all[:].rearrange("p (c r) -> p c r", c=4)

                # Software-pipelined emission: per iteration the PE stream is
                # [g2-part_t, h1-transpose_t, wih2_t, g1_{t+1}, filler] before
                # the cell2-gated h2-transpose, so PE never head-of-line
                # blocks on a cell chain for long.
                xih_t = xp.tile([128, H], BF16, tag="xih")
                nc.sync.dma_start(xih_t[:], xih_dram[0])
                dummy(512, 30)   # bridge the AllGather wait, stay warm
                ps_g1 = rps.tile([128, H], F32, tag="g", space="PSUM")
                nc.tensor.matmul(ps_g1[:], ident_bf[:], xih_t[:],
                                 start=True, stop=False, skip_group_check=True)
                nc.tensor.matmul(ps_g1[:], ident_bf[:], xeall_bf[:],
                                 start=False, stop=True, skip_group_check=True)

                h1, c1 = cell(ps_g1, c1, "1")

                for t in range(tt):
                    # ---- h1 pipeline: h1T(t) -> g1(t+1) -> cell1(t+1) ----
                    dummy(256, 1)
                    h1T = rp.tile([128, 128], BF16, tag="h1T")
                    transpose_state(h1, [h1T[:]])
                    if t + 1 < tt:
                        xih_t = xp.tile([128, H], BF16, tag="xih")
                        nc.sync.dma_start(xih_t[:], xih_dram[t + 1])
                        ps_g1 = rps.tile([128, H], F32, tag="g", space="PSUM")
                        nc.tensor.matmul(ps_g1[:], ident_bf[:], xih_t[:],
                                         start=True, stop=False,
                                         skip_group_check=True)
                        nc.tensor.matmul(ps_g1[:], ident_bf[:],
                                         xeall_bf[:],
                                         start=False, stop=False,
                                         skip_group_check=True)
                        gemm_block(ps_g1, h1T, lambda kc: 32 * kc, whh1_sb,
                                   True)
                        h1_next, c1 = cell(ps_g1, c1, "1")
                    else:
                        h1_next = None

                    # ---- h2 pipeline: h2T(t-1) -> g2(t) -> cell2(t) ----
                    if t > 0:
                        dummy(256, 1)
                        tpv = tps.tile([128, 128], BF16, tag="tps",
                                       space="PSUM")
                        for kc in range(4):
                            nc.tensor.transpose(
                                tpv[:, 32 * kc:32 * kc + 32],
                                h2[:, 128 * kc:128 * kc + 128],
                                ident_bf[0:32, 0:32])
                        nc.vector.tensor_copy(
                            h2T_view[:, :, 32 * (t - 1):32 * (t - 1) + 32],
                            tpv[:].rearrange("p (c r) -> p c r", c=4))

                    ps_g2 = rps.tile([128, H], F32, tag="g", space="PSUM")
                    nc.tensor.matmul(ps_g2[:], ident_bf[:], b2_sb[:],
                                     start=True, stop=False,
                                     skip_group_check=True)
                    if t > 0:
                        gemm_block(ps_g2, h2T_all,
                                   lambda kc, _t=t: kc * RPAD + 32 * (_t - 1),
                                   whh2_sb, False)
                    gemm_block(ps_g2, h1T, lambda kc: 32 * kc, wih2_sb, True)
                    h2, c2 = cell(ps_g2, c2, "2")

                    # ---- fillers: xdec chunks early, vocab units after ----
                    if XDEC_PRE + t < NM:
                        xdec_chunk(XDEC_PRE + t, tps, "tps", vps, "vps")
                    avail = min(NM, max(0, (t - 4) // 4 + 1))
                    quota = min(len(vunits), max(0, 3 * (t - 3)))
                    while (vemitted < quota
                           and vemitted < len(vunits)
                           and vunits[vemitted][0] < avail):
                        vocab_unit(*vunits[vemitted])
                        vemitted += 1

                    h1 = h1_next

                # epilogue: last h2 transpose, then drain vocab
                tpv = tps.tile([128, 128], BF16, tag="tps", space="PSUM")
                for kc in range(4):
                    nc.tensor.transpose(tpv[:, 32 * kc:32 * kc + 32],
                                        h2[:, 128 * kc:128 * kc + 128],
                                        ident_bf[0:32, 0:32])
                nc.vector.tensor_copy(
                    h2T_view[:, :, 32 * (tt - 1):32 * (tt - 1) + 32],
                    tpv[:].rearrange("p (c r) -> p c r", c=4))

                while vemitted < len(vunits):
                    vocab_unit(*vunits[vemitted])
                    vemitted += 1

    nc.compile()
    return nc


# =====================================================================
# Host side
# =====================================================================

def _bf16(a):
    import ml_dtypes
    return np.ascontiguousarray(np.asarray(a, dtype=np.float32).astype(
        ml_dtypes.bfloat16))


def _chunk(a):
    """(c*128, X) -> (128, c*X): partition-chunked layout for SBUF tiles."""
    c = a.shape[0] // 128
    return np.ascontiguousarray(
        a.reshape(c, 128, -1).transpose(1, 0, 2).reshape(128, -1))


def host_prep(inputs, tt=TT, vs=VS):
    """Build per-core input maps from the full problem inputs."""
    R = tt * B
    NM = math.ceil(R / 128)
    f32 = lambda a: np.ascontiguousarray(np.asarray(a), dtype=np.float32)
    # gate permutation [i, f, o, g]
    perm = np.concatenate([np.arange(0, H), np.arange(H, 2 * H),
                           np.arange(3 * H, 4 * H), np.arange(2 * H, 3 * H)])

    src = np.asarray(inputs["src"])
    trg = np.asarray(inputs["trg"])

    w_ih1 = f32(inputs["w_ih1"])[perm]
    b1 = (f32(inputs["b_ih1"]) + f32(inputs["b_hh1"]))[perm][None, :]
    b2 = (f32(inputs["b_ih2"]) + f32(inputs["b_hh2"]))[perm]
    b2pack = np.ascontiguousarray(
        np.broadcast_to(b2.reshape(4, 1, H), (4, 32, H)).reshape(128, H))

    shared = {
        "enc_emb": _bf16(inputs["enc_emb"]),
        "dec_emb": _bf16(inputs["dec_emb"]),
        "bconv": np.ascontiguousarray(
            np.stack([f32(inputs[f"conv_b{k}"]).reshape(2, 128)[fc]
                      for fc in range(2) for k in FS], axis=1)),
        "fc1T": _bf16(_chunk(f32(inputs["fc1_w"]).T)),
        "fc1b": _bf16(f32(inputs["fc1_b"])[None, :]),
        "fc2T": _bf16(_chunk(f32(inputs["fc2_w"]).T)),
        "fc2b": _bf16(f32(inputs["fc2_b"])[None, :]),
        "WdT": _bf16(_chunk(np.ascontiguousarray(w_ih1[:, :E].T))),
        "WeT": _bf16(_chunk(np.ascontiguousarray(w_ih1[:, E:].T))),
        "b1row": _bf16(b1), "b2pack": _bf16(b2pack),
        "whh1T": _bf16(_chunk(np.ascontiguousarray(f32(inputs["w_hh1"])[perm].T))),
        "wih2T": _bf16(_chunk(np.ascontiguousarray(f32(inputs["w_ih2"])[perm].T))),
        "whh2T": _bf16(_chunk(np.ascontiguousarray(f32(inputs["w_hh2"])[perm].T))),
    }
    for k in FS:
        A = f32(inputs[f"conv_w{k}"]).transpose(2, 1, 0)   # (k, E, F)
        A = A.reshape(k, 2, 128, 2, 128).transpose(0, 1, 3, 2, 4)
        shared[f"wconv{k}"] = _bf16(_chunk(A.reshape(k * 4 * 128, 128)))

    dtoks = trg[:, :tt].T.reshape(-1).astype(np.int32)
    dtoks = np.concatenate([dtoks, np.zeros(NM * 128 - R, np.int32)])
    dec_idx = np.ascontiguousarray(dtoks.reshape(NM, 128).T)

    owT_full = np.ascontiguousarray(f32(inputs["out_w"]).T)   # (H, V)
    ob_full = f32(inputs["out_b"])

    in_maps = []
    for c in range(NCORES):
        stoks = src[BL * c: BL * (c + 1)].reshape(-1).astype(np.int32)
        m = dict(shared)
        m["src_idx"] = np.ascontiguousarray(stoks.reshape(-1, 128).T)
        m["dec_idx"] = dec_idx
        m["owT"] = _bf16(_chunk(np.ascontiguousarray(
            owT_full[:, vs * c: vs * (c + 1)])))
        m["obcast"] = np.ascontiguousarray(np.broadcast_to(
            ob_full[None, vs * c: vs * (c + 1)], (128, vs)),
            dtype=np.float32)
        in_maps.append(m)
    return in_maps


def assemble(results, tt=TT, vs=VS):
    """Gather per-core logit shards -> full (B, T, V) output."""
    out = np.zeros((B, T, V), dtype=np.float32)
    for c, res in enumerate(results):
        sh = np.asarray(res["logits_sh"]).reshape(tt, B, vs)
        out[:, 1:1 + tt, vs * c: vs * (c + 1)] = sh.transpose(1, 0, 2)
    return out


_CACHE = {}


def kernel(**inputs):
    if "nc" not in _CACHE:
        _CACHE["nc"] = build()
    nc = _CACHE["nc"]
    from concourse.bass_utils import run_bass_kernel_spmd
    in_maps = host_prep(inputs)
    res = run_bass_kernel_spmd(nc, in_maps, core_ids=list(range(NCORES)))
    return assemble(res.results)


# revision 18
# speedup vs baseline: 1.1422x; 1.1422x over previous
"""CNNSummarizer (CNN encoder + 2-layer LSTM decoder + vocab projection) on 8 trn2 cores.

Sharding:
  - encoder: data-parallel over batch (4 batches per core); one AllGather of the
    per-batch encoder contribution to the LSTM-1 input preactivation (32KB).
  - LSTM recurrence: replicated on all 8 cores (small-collective latency makes
    per-step sharding a loss).
  - vocab projection (H -> V GEMM): column-sharded, 4000 vocab per core.

v2 changes vs the original working version:
  - all-bf16 data path (embeddings pre-cast on host, conv / fc / Xdec / bypass
    matmuls bf16) -- avoids fp32-HIGH 2-pass matmuls and halves DMA traffic.
  - recurrence + vocab weights DMA'd at program start (pools coexist with the
    encoder pools) so the recurrence never waits on weight loads.
  - index DMAs issued first so embedding gathers start immediately.
  - dec-token gathers all issued before the AllGather on the gpsimd queue.
  - Xdec m-chunks 0..3 computed during the AllGather window; chunks 4..15
    interleaved into recurrence steps 0..11 as PE filler work.
  - greedy vocab-unit emission (fills PE gaps while LSTM cell chains run on
    ACT/DVE, keeping the PE HAM clock-gate warm).

The LSTM input preactivations X@w_ih1 are precomputed for all steps (teacher
forcing) as one big GEMM into a DRAM buffer packed as (t, 32*gateblock+batch,
512); the recurrent h@w_hh GEMMs then run as 4-way column-tiled matmuls
(stationary = h^T K-chunk, streaming = weight rows) producing gates directly in
a (gateblock*32+b, d) PSUM layout, gate order host-permuted to [i, f, o, g] so
sigmoid runs as one 96-partition op.
"""

import math
from contextlib import ExitStack

import numpy as np

import concourse.bacc as bacc
import concourse.bass as bass
import concourse.mybir as mybir
import concourse.tile as tile
from concourse.masks import make_identity

V, E, H, F = 32000, 256, 512, 256
FS = (3, 4, 5)
B, S, T = 32, 512, 64
NCORES = 8
BL = B // NCORES          # batches per core
VS = V // NCORES          # vocab shard per core
TT = T - 1                # decode steps actually computed
G4 = 4 * H                # 2048 gates

dt = mybir.dt
F32 = dt.float32
BF16 = dt.bfloat16
AF = mybir.ActivationFunctionType
ALU = mybir.AluOpType
AX = mybir.AxisListType


def build(tt=TT, vs=VS, trace_sim=False):
    """Build the per-core program. All 8 cores run the same NEFF; sharding comes
    from per-core input values."""
    R = tt * B                       # rows of the (t, b) decode matrix
    NM = math.ceil(R / 128)          # m-chunks of decode rows
    NCH = NM                         # dec-token gather chunks (128 tokens each)
    RPAD = NM * 128
    NV = math.ceil(vs / 512)
    XDEC_PRE = 12                    # Xdec chunks computed before the recurrence

    nc = bacc.Bacc("TRN2", target_bir_lowering=False, debug=False,
                   num_devices=NCORES)

    def inp(name, shape, dtype=BF16):
        return nc.dram_tensor(name, list(shape), dtype, kind="ExternalInput").ap()

    src_idx = inp("src_idx", (128, (BL * S) // 128), dt.int32)
    dec_idx = inp("dec_idx", (128, NCH), dt.int32)
    enc_emb = inp("enc_emb", (V, E))          # bf16 (host pre-cast)
    dec_emb = inp("dec_emb", (V, E))          # bf16
    wconv = {k: inp(f"wconv{k}", (128, k * 4 * 128)) for k in FS}
    bconv = inp("bconv", (128, 2 * len(FS)), F32)   # col = fc*3 + k_idx
    fc1T = inp("fc1T", (128, 6 * H))
    fc1b = inp("fc1b", (1, H))
    fc2T = inp("fc2T", (128, 4 * H))
    fc2b = inp("fc2b", (1, H))
    WdT = inp("WdT", (128, 2 * G4))
    WeT = inp("WeT", (128, 4 * G4))
    b1row = inp("b1row", (1, G4))
    b2pack = inp("b2pack", (128, H))
    whh1T = inp("whh1T", (128, 4 * G4))
    wih2T = inp("wih2T", (128, 4 * G4))
    whh2T = inp("whh2T", (128, 4 * G4))
    owT = inp("owT", (128, 4 * vs))
    obcast_in = inp("obcast", (128, vs), F32)

    out_dram = nc.dram_tensor("logits_sh", [R, vs], F32,
                              kind="ExternalOutput").ap()

    with tile.TileContext(nc, trace_sim=trace_sim) as tc:
        with ExitStack() as ctx:
            dram = ctx.enter_context(tc.tile_pool(name="dram", bufs=1,
                                                  space="DRAM"))
            xih_dram = dram.tile([tt, 128, H], BF16)
            cc_in = dram.tile([BL, G4], F32)
            cc_out = dram.tile([B, G4], F32, addr_space="Shared")

            # ---- constants ----
            const = ctx.enter_context(tc.tile_pool(name="const", bufs=1))
            identF = const.tile([128, 128], F32)
            make_identity(nc, identF[:])
            ident_bf = const.tile([128, 128], BF16)
            nc.vector.tensor_copy(ident_bf[:], identF[:])
            scrF = const.tile([128, 128], F32)
            nc.vector.memset(scrF[:], 0.0)
            zpad = const.tile([128, 8], BF16)
            nc.vector.tensor_copy(zpad[:], scrF[:, 0:8])
            nc.vector.memset(scrF[0:1, :], 1.0)
            ones_bf = const.tile([1, 128], BF16)
            nc.vector.tensor_copy(ones_bf[:], scrF[0:1, :])

            # ---- persistent recurrence weights: DMA'd up front ----
            rw = ctx.enter_context(tc.tile_pool(name="rw", bufs=1))
            whh1_sb = rw.tile([128, 4 * G4], BF16)
            wih2_sb = rw.tile([128, 4 * G4], BF16)
            whh2_sb = rw.tile([128, 4 * G4], BF16)
            b2_sb = rw.tile([128, H], BF16)
            xeall_f32 = rw.tile([128, H], F32)
            xeall_bf = rw.tile([128, H], BF16)

            # h2^T lives across recurrence + vocab phases
            big = ctx.enter_context(tc.tile_pool(name="big", bufs=1))
            h2T_all = big.tile([128, 4 * RPAD], BF16)   # [kc] blocks of h2^T

            # ---- mid-lived: dec-token gather + Xdec staging ----
            midp = ctx.enter_context(tc.tile_pool(name="midp", bufs=1))
            idx_d_sb = midp.tile([128, NCH], dt.int32)
            dembT_sb = midp.tile([128, 2 * RPAD], BF16)   # [ec] blocks
            WdT_sb = midp.tile([128, 2 * G4], BF16)
            gpool_dec = ctx.enter_context(tc.tile_pool(name="gpool_dec",
                                                       bufs=NM))
            xdpool = ctx.enter_context(tc.tile_pool(name="xdpool", bufs=4))

            # =========================================================
            # Phase 1: encoder (my BL batches) + Xdec chunks 0..3
            # =========================================================
            p1 = ExitStack()
            encp = p1.enter_context(tc.tile_pool(name="encp", bufs=1))
            gpool = p1.enter_context(tc.tile_pool(name="gpool", bufs=16))
            tpp = p1.enter_context(tc.tile_pool(name="tpp", bufs=2,
                                                space="PSUM"))
            cps = p1.enter_context(tc.tile_pool(name="cps", bufs=3,
                                                space="PSUM"))
            fps = p1.enter_context(tc.tile_pool(name="fps", bufs=2,
                                                space="PSUM"))

            wconv_sb = {k: encp.tile([128, k * 4 * 128], BF16,
                                     name=f"wconv{k}_sb") for k in FS}
            bconv_sb = encp.tile([128, 2 * len(FS)], F32)
            fc1T_sb = encp.tile([128, 6 * H], BF16)
            fc2T_sb = encp.tile([128, 4 * H], BF16)
            fc1b_sb = encp.tile([1, H], BF16)
            fc2b_sb = encp.tile([1, H], BF16)
            WeT_sb = encp.tile([128, 4 * G4], BF16)
            b1_sb = encp.tile([1, G4], BF16)
            idx_s_sb = encp.tile([128, (BL * S) // 128], dt.int32)

            XPAD = BL * (S + 8)
            SEG = S + 8
            xT_sb = encp.tile([128, 2 * XPAD], BF16)        # [ec] blocks

            # ---- DMA issue order = priority order ----
            nc.sync.dma_start(idx_s_sb[:], src_idx)
            nc.sync.dma_start(idx_d_sb[:], dec_idx)
            for k in FS:
                nc.sync.dma_start(wconv_sb[k][:], wconv[k])
            nc.sync.dma_start(bconv_sb[:], bconv)
            nc.sync.dma_start(fc1T_sb[:], fc1T)
            nc.sync.dma_start(fc1b_sb[:], fc1b)
            nc.sync.dma_start(fc2T_sb[:], fc2T)
            nc.sync.dma_start(fc2b_sb[:], fc2b)
            nc.sync.dma_start(WdT_sb[:], WdT)
            nc.sync.dma_start(WeT_sb[:], WeT)
            nc.sync.dma_start(b1_sb[:], b1row)
            nc.sync.dma_start(b2_sb[:], b2pack)
            nc.sync.dma_start(whh1_sb[:], whh1T)
            nc.sync.dma_start(wih2_sb[:], wih2T)
            nc.sync.dma_start(whh2_sb[:], whh2T)

            def evict(dst, src, parity):
                if parity % 2 == 0:
                    nc.vector.tensor_copy(dst, src)
                else:
                    nc.scalar.copy(dst, src)

            # ---- src gathers (gpsimd queue) ----
            gts = []
            for b in range(BL):
                for ch in range(4):
                    gt = gpool.tile([128, E], BF16, tag="gath")
                    nc.gpsimd.indirect_dma_start(
                        out=gt[:], out_offset=None, in_=enc_emb,
                        in_offset=bass.IndirectOffsetOnAxis(
                            ap=idx_s_sb[:, 4 * b + ch:4 * b + ch + 1],
                            axis=0))
                    gts.append(gt)
            # ---- dec gathers, all prefetched before the AllGather ----
            gtds = []
            for m in range(NM):
                gt = gpool_dec.tile([128, E], BF16, tag="gathd")
                nc.gpsimd.indirect_dma_start(
                    out=gt[:], out_offset=None, in_=dec_emb,
                    in_offset=bass.IndirectOffsetOnAxis(
                        ap=idx_d_sb[:, m:m + 1], axis=0))
                gtds.append(gt)

            pooled = encp.tile([128, 6 * BL], BF16)

            def conv_batch(b):
                for ki, k in enumerate(FS):
                    for fc in range(2):
                        ps = cps.tile([128, 512], F32, tag="conv",
                                      space="PSUM")
                        first = True
                        for j in range(k):
                            for ec in range(2):
                                lhs = wconv_sb[k][
                                    :, (j * 4 + ec * 2 + fc) * 128:
                                    (j * 4 + ec * 2 + fc) * 128 + 128]
                                rhs = xT_sb[:, ec * XPAD + SEG * b + j:
                                            ec * XPAD + SEG * b + j + 512]
                                nc.tensor.matmul(
                                    ps[:], lhs, rhs, start=first,
                                    stop=(j == k - 1 and ec == 1))
                                first = False
                        kc = ki * 2 + fc
                        nc.vector.tensor_reduce(
                            pooled[:, BL * kc + b: BL * kc + b + 1],
                            ps[:, 0:S - k + 1], axis=AX.X, op=ALU.max)

            # conv staircase: transpose batch b's gathers, then its convs
            for b in range(BL):
                for ch in range(4):
                    gt = gts[4 * b + ch]
                    for ec in range(2):
                        tp = tpp.tile([128, 128], BF16, tag="tp",
                                      space="PSUM")
                        nc.tensor.transpose(
                            tp[:], gt[:, 128 * ec:128 * ec + 128],
                            ident_bf[:])
                        evict(xT_sb[:, ec * XPAD + SEG * b + 128 * ch:
                                    ec * XPAD + SEG * b + 128 * ch + 128],
                              tp[:], ch + ec)
                for ec in range(2):
                    nc.vector.tensor_copy(
                        xT_sb[:, ec * XPAD + SEG * b + S:
                              ec * XPAD + SEG * (b + 1)], zpad[:])
                conv_batch(b)

            for ki in range(len(FS)):
                for fc in range(2):
                    kc = ki * 2 + fc
                    nc.scalar.activation(
                        pooled[:, BL * kc: BL * kc + BL],
                        pooled[:, BL * kc: BL * kc + BL],
                        AF.Relu, bias=bconv_sb[:, fc * 3 + ki: fc * 3 + ki + 1])

            # ---- fc1 -> relu -> fc2 -> Xenc -> AllGather ----
            ps1 = fps.tile([BL, H], F32, tag="f", space="PSUM")
            for kc in range(6):
                nc.tensor.matmul(ps1[:], pooled[:, BL * kc: BL * kc + BL],
                                 fc1T_sb[:, H * kc: H * kc + H],
                                 start=(kc == 0), stop=False)
            nc.tensor.matmul(ps1[:], ones_bf[0:1, 0:BL], fc1b_sb[:],
                             start=False, stop=True)
            h1e = encp.tile([BL, H], BF16)
            nc.scalar.activation(h1e[:], ps1[:], AF.Relu)

            h1eT = encp.tile([128, 4 * BL], BF16)
            for kc in range(4):
                tp = tpp.tile([128, 128], BF16, tag="tp", space="PSUM")
                nc.tensor.transpose(tp[0:128, 0:BL],
                                    h1e[:, 128 * kc:128 * kc + 128],
                                    ident_bf[0:BL, 0:BL])
                nc.vector.tensor_copy(h1eT[:, BL * kc:BL * kc + BL],
                                      tp[0:128, 0:BL])

            ps2 = fps.tile([BL, H], F32, tag="f", space="PSUM")
            for kc in range(4):
                nc.tensor.matmul(ps2[:], h1eT[:, BL * kc:BL * kc + BL],
                                 fc2T_sb[:, H * kc:H * kc + H],
                                 start=(kc == 0), stop=False)
            nc.tensor.matmul(ps2[:], ones_bf[0:1, 0:BL], fc2b_sb[:],
                             start=False, stop=True)
            enc_sb = encp.tile([BL, H], BF16)
            nc.vector.tensor_copy(enc_sb[:], ps2[:])

            encT = encp.tile([128, 4 * BL], BF16)
            for kc in range(4):
                tp = tpp.tile([128, 128], BF16, tag="tp", space="PSUM")
                nc.tensor.transpose(tp[0:128, 0:BL],
                                    enc_sb[:, 128 * kc:128 * kc + 128],
                                    ident_bf[0:BL, 0:BL])
                nc.vector.tensor_copy(encT[:, BL * kc:BL * kc + BL],
                                      tp[0:128, 0:BL])

            xe_sb = encp.tile([BL, G4], F32)
            for n in range(4):
                ps = fps.tile([BL, 512], F32, tag="f", space="PSUM")
                for kc in range(4):
                    nc.tensor.matmul(
                        ps[:], encT[:, BL * kc:BL * kc + BL],
                        WeT_sb[:, kc * G4 + 512 * n:
                               kc * G4 + 512 * n + 512],
                        start=(kc == 0), stop=False)
                nc.tensor.matmul(ps[:], ones_bf[0:1, 0:BL],
                                 b1_sb[:, 512 * n:512 * n + 512],
                                 start=False, stop=True)
                nc.vector.tensor_copy(xe_sb[:, 512 * n:512 * n + 512], ps[:])
            # gpsimd-queue DMA: keeps the xe-gated transfer off the sync
            # queue so Xdec stores flow during the AllGather window
            nc.gpsimd.dma_start(cc_in[:], xe_sb[:])

            nc.gpsimd.collective_compute(
                "AllGather", ALU.bypass,
                replica_groups=[list(range(NCORES))],
                ins=[cc_in.opt()], outs=[cc_out.opt()])

            # ---- Xdec chunk machinery (shared by phase 1 + recurrence) ----
            def xdec_chunk(m, tp_pool, tp_tag, ps_pool, ps_tag):
                tm = min(4, tt - 4 * m)
                Mm = 32 * tm
                gt = gtds[m]
                for ec in range(2):
                    tp = tp_pool.tile([128, 128], BF16, tag=tp_tag,
                                      space="PSUM")
                    nc.tensor.transpose(
                        tp[:], gt[:, 128 * ec:128 * ec + 128], ident_bf[:])
                    evict(dembT_sb[:, ec * RPAD + 128 * m:
                                   ec * RPAD + 128 * m + 128],
                          tp[:], m + ec)
                xd_sb = xdpool.tile([128, G4], BF16, tag="xd_sb")
                for n in range(4):
                    ps = ps_pool.tile([128, 512], F32, tag=ps_tag,
                                      space="PSUM")
                    for ec in range(2):
                        nc.tensor.matmul(
                            ps[0:Mm, :],
                            dembT_sb[:, ec * RPAD + 128 * m:
                                     ec * RPAD + 128 * m + Mm],
                            WdT_sb[:, ec * G4 + 512 * n:
                                   ec * G4 + 512 * n + 512],
                            start=(ec == 0), stop=(ec == 1))
                    evict(xd_sb[0:Mm, 512 * n:512 * n + 512],
                          ps[0:Mm, :], m + n)
                for tau in range(tm):
                    dst = xih_dram[4 * m + tau].rearrange(
                        "(j b) d -> b j d", j=4)
                    nc.sync.dma_start(dst, xd_sb[32 * tau:32 * tau + 32, :])

            # chunks 0..XDEC_PRE-1 overlap the AllGather
            for m in range(XDEC_PRE):
                xdec_chunk(m, tpp, "tp", cps, "conv")

            p1.close()

            # =========================================================
            # Phase 2: recurrence with packed gate PSUM, col-tiled GEMMs
            # gate-block order [i, f, o, g] on psum partitions [0:32,...]
            # =========================================================
            with ExitStack() as p2:
                vocabw = p2.enter_context(tc.tile_pool(name="vocabw", bufs=1))
                owT_sb = vocabw.tile([128, 4 * vs], BF16)
                obcast = vocabw.tile([128, vs], F32)
                # vector-queue DMAs: the sync queue head-blocks on the
                # AllGather-dependent loads below; these must not wait.
                nc.scalar.dma_start(owT_sb[:], owT)
                nc.scalar.dma_start(obcast[:], obcast_in)

                rp = p2.enter_context(tc.tile_pool(name="rp", bufs=2))
                xp = p2.enter_context(tc.tile_pool(name="xp", bufs=3))
                vo = p2.enter_context(tc.tile_pool(name="vo", bufs=3))
                rps = p2.enter_context(tc.tile_pool(name="rps", bufs=2,
                                                    space="PSUM"))
                dps = p2.enter_context(tc.tile_pool(name="dps", bufs=1,
                                                    space="PSUM"))
                tps = p2.enter_context(tc.tile_pool(name="tps", bufs=1,
                                                    space="PSUM"))
                vps = p2.enter_context(tc.tile_pool(name="vps", bufs=3,
                                                    space="PSUM"))

                # Xenc contribution, repacked to the (32*gateblock+b, d) layout
                nc.sync.dma_start(xeall_f32[:],
                                  cc_out.rearrange("b (j d) -> j b d", j=4))
                nc.vector.tensor_copy(xeall_bf[:], xeall_f32[:])

                dummy_ps = dps.tile([128, 512], F32, tag="d", space="PSUM")

                def dummy(n=256, count=1):
                    """Keep-warm matmuls: PE-busy filler during cell-chain
                    stalls so the HAM clock gate stays open."""
                    for _ in range(count):
                        nc.tensor.matmul(dummy_ps[:, 0:n], ident_bf[:],
                                         b2_sb[:, 0:n], start=True, stop=True,
                                         skip_group_check=True)

                c1 = rp.tile([64, H], BF16, tag="c1")
                nc.vector.memset(c1[32:64, :], 0.0)
                c2 = rp.tile([64, H], BF16, tag="c2")
                nc.vector.memset(c2[32:64, :], 0.0)

                def cell(ps_g, c_prev, tag):
                    """LSTM cell from packed-gate psum (128, H) -> (h, c_new).

                    Gate blocks keep their partition homes: i/f/o from one
                    96-partition sigmoid, tanh(g) lands at [0:32]. c lives at
                    [32:64]. All elementwise math in bf16.
                    """
                    sig = rp.tile([96, H], BF16, tag=f"sig{tag}")
                    nc.scalar.activation(sig[:], ps_g[0:96, :], AF.Sigmoid)
                    tg = rp.tile([32, H], BF16, tag=f"tg{tag}")
                    nc.scalar.activation(tg[:], ps_g[96:128, :], AF.Tanh)
                    c_new = rp.tile([64, H], BF16, tag=f"c{tag}")
                    nc.vector.tensor_mul(c_new[32:64, :], sig[32:64, :],
                                         c_prev[32:64, :])
                    m1 = rp.tile([64, H], BF16, tag=f"m1{tag}")
                    nc.vector.tensor_mul(m1[32:64, :], sig[0:32, :], tg[:])
                    nc.vector.tensor_add(c_new[32:64, :], m1[32:64, :],
                                         c_new[32:64, :])
                    th = rp.tile([96, H], BF16, tag=f"th{tag}")
                    nc.scalar.activation(th[64:96, :], c_new[32:64, :], AF.Tanh)
                    h = rp.tile([32, H], BF16, tag=f"h{tag}")
                    nc.vector.tensor_mul(h[:], sig[64:96, :], th[64:96, :])
                    return h, c_new

                def transpose_state(h, dsts):
                    tp = tps.tile([128, 128], BF16, tag="tps",
                                  space="PSUM")
                    for kc in range(4):
                        nc.tensor.transpose(tp[:, 32 * kc:32 * kc + 32],
                                            h[:, 128 * kc:128 * kc + 128],
                                            ident_bf[0:32, 0:32])
                    for dst in dsts:
                        nc.vector.tensor_copy(dst, tp[:])
                    return tp

                def gemm_block(ps, stat, stat_base, w_sb, final):
                    for kc in range(4):
                        for j in range(4):
                            nc.tensor.matmul(
                                ps[32 * j:32 * j + 32, :],
                                stat[:, stat_base(kc): stat_base(kc) + 32],
                                w_sb[:, kc * G4 + 512 * j:
                                     kc * G4 + 512 * j + 512],
                                start=False,
                                stop=(final and kc == 3 and j == 3),
                                skip_group_check=True,
                                tile_position=(0, 32 * j))

                def vocab_unit(m, n):
                    """One (row-chunk, vocab-tile) unit of the output GEMM."""
                    Mm = min(128, R - 128 * m)
                    nw = min(512, vs - 512 * n)
                    ps = vps.tile([128, 512], F32, tag="vps", space="PSUM")
                    for kc in range(4):
                        nc.tensor.matmul(
                            ps[0:Mm, 0:nw],
                            h2T_all[:, kc * RPAD + 128 * m:
                                    kc * RPAD + 128 * m + Mm],
                            owT_sb[:, kc * vs + 512 * n: kc * vs + 512 * n + nw],
                            start=(kc == 0), stop=(kc == 3))
                    ob = vo.tile([128, 512], F32, tag="ob")
                    nc.vector.tensor_add(ob[0:Mm, 0:nw], ps[0:Mm, 0:nw],
                                         obcast[0:Mm, 512 * n:512 * n + nw])
                    nc.sync.dma_start(
                        out_dram[128 * m:128 * m + Mm, 512 * n:512 * n + nw],
                        ob[0:Mm, 0:nw])

                vunits = [(m, n) for m in range(NM) for n in range(NV)]
                vemitted = 0

                h2T_view = h2T_all[:].rearrange("p (c r) -> p c r", c=4)

                # Software-pipelined emission: per iteration the PE stream is
                # [g2-part_t, h1-transpose_t, wih2_t, g1_{t+1}, filler] before
                # the cell2-gated h2-transpose, so PE never head-of-line
                # blocks on a cell chain for long.
                xih_t = xp.tile([128, H], BF16, tag="xih")
                nc.sync.dma_start(xih_t[:], xih_dram[0])
                dummy(512, 30)   # bridge the AllGather wait, stay warm
                ps_g1 = rps.tile([128, H], F32, tag="g", space="PSUM")
                nc.tensor.matmul(ps_g1[:], ident_bf[:], xih_t[:],
                                 start=True, stop=False, skip_group_check=True)
                nc.tensor.matmul(ps_g1[:], ident_bf[:], xeall_bf[:],
                                 start=False, stop=True, skip_group_check=True)

                h1, c1 = cell(ps_g1, c1, "1")

                for t in range(tt):
                    # ---- h1 pipeline: h1T(t) -> g1(t+1) -> cell1(t+1) ----
                    dummy(256, 1)
                    h1T = rp.tile([128, 128], BF16, tag="h1T")
                    transpose_state(h1, [h1T[:]])
                    if t + 1 < tt:
                        xih_t = xp.tile([128, H], BF16, tag="xih")
                        nc.sync.dma_start(xih_t[:], xih_dram[t + 1])
                        ps_g1 = rps.tile([128, H], F32, tag="g", space="PSUM")
                        nc.tensor.matmul(ps_g1[:], ident_bf[:], xih_t[:],
                                         start=True, stop=False,
                                         skip_group_check=True)
                        nc.tensor.matmul(ps_g1[:], ident_bf[:],
                                         xeall_bf[:],
                                         start=False, stop=False,
                                         skip_group_check=True)
                        gemm_block(ps_g1, h1T, lambda kc: 32 * kc, whh1_sb,
                                   True)
                        h1_next, c1 = cell(ps_g1, c1, "1")
                    else:
                        h1_next = None

                    # ---- h2 pipeline: h2T(t-1) -> g2(t) -> cell2(t) ----
                    if t > 0:
                        dummy(256, 1)
                        tpv = tps.tile([128, 128], BF16, tag="tps",
                                       space="PSUM")
                        for kc in range(4):
                            nc.tensor.transpose(
                                tpv[:, 32 * kc:32 * kc + 32],
                                h2[:, 128 * kc:128 * kc + 128],
                                ident_bf[0:32, 0:32])
                        nc.vector.tensor_copy(
                            h2T_view[:, :, 32 * (t - 1):32 * (t - 1) + 32],
                            tpv[:].rearrange("p (c r) -> p c r", c=4))

                    ps_g2 = rps.tile([128, H], F32, tag="g", space="PSUM")
                    nc.tensor.matmul(ps_g2[:], ident_bf[:], b2_sb[:],
                                     start=True, stop=False,
                                     skip_group_check=True)
                    if t > 0:
                        gemm_block(ps_g2, h2T_all,
                                   lambda kc, _t=t: kc * RPAD + 32 * (_t - 1),
                                   whh2_sb, False)
                    gemm_block(ps_g2, h1T, lambda kc: 32 * kc, wih2_sb, True)
                    h2, c2 = cell(ps_g2, c2, "2")

                    # ---- fillers: xdec chunks early, vocab units after ----
                    if XDEC_PRE + t < NM:
                        xdec_chunk(XDEC_PRE + t, tps, "tps", vps, "vps")
                    avail = min(NM, max(0, (t - 4) // 4 + 1))
                    quota = min(len(vunits), max(0, 3 * (t - 3)))
                    while (vemitted < quota
                           and vemitted < len(vunits)
                           and vunits[vemitted][0] < avail):
                        vocab_unit(*vunits[vemitted])
                        vemitted += 1

                    h1 = h1_next

                # epilogue: last h2 transpose, then drain vocab
                tpv = tps.tile([128, 128], BF16, tag="tps", space="PSUM")
                for kc in range(4):
                    nc.tensor.transpose(tpv[:, 32 * kc:32 * kc + 32],
                                        h2[:, 128 * kc:128 * kc + 128],
                                        ident_bf[0:32, 0:32])
                nc.vector.tensor_copy(
                    h2T_view[:, :, 32 * (tt - 1):32 * (tt - 1) + 32],
                    tpv[:].rearrange("p (c r) -> p c r", c=4))

                while vemitted < len(vunits):
                    vocab_unit(*vunits[vemitted])
                    vemitted += 1

    nc.compile()
    return nc


# =====================================================================
# Host side
# =====================================================================

def _bf16(a):
    import ml_dtypes
    return np.ascontiguousarray(np.asarray(a, dtype=np.float32).astype(
        ml_dtypes.bfloat16))


def _chunk(a):
    """(c*128, X) -> (128, c*X): partition-chunked layout for SBUF tiles."""
    c = a.shape[0] // 128
    return np.ascontiguousarray(
        a.reshape(c, 128, -1).transpose(1, 0, 2).reshape(128, -1))


def host_prep(inputs, tt=TT, vs=VS):
    """Build per-core input maps from the full problem inputs."""
    R = tt * B
    NM = math.ceil(R / 128)
    f32 = lambda a: np.ascontiguousarray(np.asarray(a), dtype=np.float32)
    # gate permutation [i, f, o, g]
    perm = np.concatenate([np.arange(0, H), np.arange(H, 2 * H),
                           np.arange(3 * H, 4 * H), np.arange(2 * H, 3 * H)])

    src = np.asarray(inputs["src"])
    trg = np.asarray(inputs["trg"])

    w_ih1 = f32(inputs["w_ih1"])[perm]
    b1 = (f32(inputs["b_ih1"]) + f32(inputs["b_hh1"]))[perm][None, :]
    b2 = (f32(inputs["b_ih2"]) + f32(inputs["b_hh2"]))[perm]
    b2pack = np.ascontiguousarray(
        np.broadcast_to(b2.reshape(4, 1, H), (4, 32, H)).reshape(128, H))

    shared = {
        "enc_emb": _bf16(inputs["enc_emb"]),
        "dec_emb": _bf16(inputs["dec_emb"]),
        "bconv": np.ascontiguousarray(
            np.stack([f32(inputs[f"conv_b{k}"]).reshape(2, 128)[fc]
                      for fc in range(2) for k in FS], axis=1)),
        "fc1T": _bf16(_chunk(f32(inputs["fc1_w"]).T)),
        "fc1b": _bf16(f32(inputs["fc1_b"])[None, :]),
        "fc2T": _bf16(_chunk(f32(inputs["fc2_w"]).T)),
        "fc2b": _bf16(f32(inputs["fc2_b"])[None, :]),
        "WdT": _bf16(_chunk(np.ascontiguousarray(w_ih1[:, :E].T))),
        "WeT": _bf16(_chunk(np.ascontiguousarray(w_ih1[:, E:].T))),
        "b1row": _bf16(b1), "b2pack": _bf16(b2pack),
        "whh1T": _bf16(_chunk(np.ascontiguousarray(f32(inputs["w_hh1"])[perm].T))),
        "wih2T": _bf16(_chunk(np.ascontiguousarray(f32(inputs["w_ih2"])[perm].T))),
        "whh2T": _bf16(_chunk(np.ascontiguousarray(f32(inputs["w_hh2"])[perm].T))),
    }
    for k in FS:
        A = f32(inputs[f"conv_w{k}"]).transpose(2, 1, 0)   # (k, E, F)
        A = A.reshape(k, 2, 128, 2, 128).transpose(0, 1, 3, 2, 4)
        shared[f"wconv{k}"] = _bf16(_chunk(A.reshape(k * 4 * 128, 128)))

    dtoks = trg[:, :tt].T.reshape(-1).astype(np.int32)
    dtoks = np.concatenate([dtoks, np.zeros(NM * 128 - R, np.int32)])
    dec_idx = np.ascontiguousarray(dtoks.reshape(NM, 128).T)

    owT_full = np.ascontiguousarray(f32(inputs["out_w"]).T)   # (H, V)
    ob_full = f32(inputs["out_b"])

    in_maps = []
    for c in range(NCORES):
        stoks = src[BL * c: BL * (c + 1)].reshape(-1).astype(np.int32)
        m = dict(shared)
        m["src_idx"] = np.ascontiguousarray(stoks.reshape(-1, 128).T)
        m["dec_idx"] = dec_idx
        m["owT"] = _bf16(_chunk(np.ascontiguousarray(
            owT_full[:, vs * c: vs * (c + 1)])))
        m["obcast"] = np.ascontiguousarray(np.broadcast_to(
            ob_full[None, vs * c: vs * (c + 1)], (128, vs)),
            dtype=np.float32)
        in_maps.append(m)
    return in_maps


def assemble(results, tt=TT, vs=VS):
    """Gather per-core logit shards -> full (B, T, V) output."""
    out = np.zeros((B, T, V), dtype=np.float32)
    for c, res in enumerate(results):
        sh = np.asarray(res["logits_sh"]).reshape(tt, B, vs)
        out[:, 1:1 + tt, vs * c: vs * (c + 1)] = sh.transpose(1, 0, 2)
    return out


_CACHE = {}


def kernel(**inputs):
    if "nc" not in _CACHE:
        _CACHE["nc"] = build()
    nc = _CACHE["nc"]
    from concourse.bass_utils import run_bass_kernel_spmd
    in_maps = host_prep(inputs)
    res = run_bass_kernel_spmd(nc, in_maps, core_ids=list(range(NCORES)))
    return assemble(res.results)
